# revision 18
# baseline (speedup 1.0000x reference)
"""AELoss (associative-embedding push/pull loss) on 8 TRN2 NeuronCores.

Strategy: data-parallel over batch — each of the 8 cores handles 4 images.
The tags tensor is huge ([B, N, 1], N = 17*256*256) but only M*K = 510
elements per image are ever read, so the kernel gathers exactly those
elements with indirect DMAs (one [128,1] gather per joint column — the HW
consumes one index per partition per instruction) and reduces on-chip:

  partitions 0..119 = persons (4 images x 30 persons), free dim = joints.
  Per-person sums after the gathers (DVE work during the gather window slows
  the Q7 SWDGE via SBUF-port contention, so the window is kept clean);
  per-image sums via PE matmul against a constant selection matrix; mean
  broadcast back via a second matmul; the pairwise push term exp(-(mi-mj)^2)
  uses an additive penalty mask and the ACT engine's accum_out row-sum.

Identities used (all exact against the reference):
  pull_pp = sum(g^2 v)/safe_cnt - mean^2
  pull    = pull_num / max(n,1)        (the n>0 guard is redundant: n=0 -> 0)
  push    = (S - n)/max(n^2-n, 1)*0.5  (n<2 guard redundant: S=n for n<=1)

The per-core output is [4, 2] = (push, pull) per image.
"""

import numpy as np

B, M, K = 32, 30, 17
N = 17 * 256 * 256
NCORES = 8
BL = B // NCORES          # images per core
P = 128
PERS = BL * M             # persons per core (120)

# packed layout (f32 columns): vis | sel | oneh | zero
C_VIS = 0
C_SEL = K                 # [17, 21)
C_ONEH = K + BL           # [21, 51)
C_ZERO = C_ONEH + M       # [51, 52)
W_PACK = C_ZERO + 1       # 52

GROUPS = [(0, 6), (6, 12), (12, 17)]   # joint-column groups for partial sums

_cache = {}


def _constants():
    p = np.arange(P)
    m = p < PERS
    sel = np.zeros((P, BL), np.float32)
    sel[p[m], (p // M)[m]] = 1.0
    selT = np.ascontiguousarray(sel.T)             # [BL, P]
    oneh = np.zeros((P, M), np.float32)
    oneh[p[m], (p % M)[m]] = 1.0
    return sel, selT, oneh


def _strip_init_barrier(nc):
    """Drop the Bass-init const-AP memsets and the all-engine barrier that
    orders them — nothing in this kernel reads the const APs (activation
    bias is passed as an explicit AP)."""
    import concourse.mybir as mybir

    bb = nc.main_func.blocks[0]
    drop = set()
    for ins in bb.instructions:
        if isinstance(ins, (mybir.InstMemset, mybir.InstDrain, mybir.InstEventSemaphore)):
            drop.add(ins.name)
    if not drop:
        return
    keep = [ins for ins in bb.instructions if ins.name not in drop]
    del bb.instructions[:]
    for ins in keep:
        bb.add_instruction(ins)


def _build():
    import concourse.bass as bass
    import concourse.bacc as bacc
    import concourse.mybir as mybir
    from concourse.tile import TileContext

    f32 = mybir.dt.float32
    i32 = mybir.dt.int32
    X = mybir.AxisListType.X
    op = mybir.AluOpType

    nc = bacc.Bacc(trn_type="TRN2")
    _strip_init_barrier(nc)
    tags_d = nc.dram_tensor("tags", [BL * N, 1], f32, kind="ExternalInput")
    idx_d = nc.dram_tensor("idx", [P, K], i32, kind="ExternalInput")
    packed_d = nc.dram_tensor("packed", [P, W_PACK], f32, kind="ExternalInput")
    selT_d = nc.dram_tensor("selT", [BL, P], f32, kind="ExternalInput")
    out_d = nc.dram_tensor("out", [BL, 2], f32, kind="ExternalOutput")

    with TileContext(nc) as tc:
        with (
            tc.tile_pool(name="sb", bufs=1) as sb,
            tc.tile_pool(name="ps", bufs=1, space="PSUM") as ps,
        ):
            # idx first, on its own HWDGE path, so the gathers start ASAP
            idx_t = sb.tile([P, K], i32)
            nc.sync.dma_start(out=idx_t[:], in_=idx_d[:])

            # gathers: one [128,1] indirect DMA per joint column
            g = sb.tile([P, K], f32)
            for k in range(K):
                nc.gpsimd.indirect_dma_start(
                    out=g[:, k:k + 1],
                    out_offset=None,
                    in_=tags_d[:],
                    in_offset=bass.IndirectOffsetOnAxis(ap=idx_t[:, k:k + 1], axis=0),
                    oob_is_err=False,
                    bounds_check=BL * N - 1,
                )

            # remaining small inputs on the scalar engine's HWDGE queue
            packed_t = sb.tile([P, W_PACK], f32)
            nc.scalar.dma_start(out=packed_t[:], in_=packed_d[:])
            vis_t = packed_t[:, C_VIS:C_VIS + K]
            oneh_t = packed_t[:, C_ONEH:C_ONEH + M]
            zero_c = packed_t[:, C_ZERO:C_ZERO + 1]
            selT_raw = sb.tile([BL, P], f32)
            nc.scalar.dma_start(out=selT_raw[:], in_=selT_d[:])

            # gather-independent prep (small, mostly off the gather window)
            sel_t = sb.tile([P, BL], f32)
            nc.vector.tensor_copy(out=sel_t[:], in_=packed_t[:, C_SEL:C_SEL + BL])
            selT_t = sb.tile([BL, P], f32)
            nc.vector.tensor_copy(out=selT_t[:], in_=selT_raw[:])

            cnt = sb.tile([P, 1], f32)
            nc.vector.reduce_sum(out=cnt[:], in_=vis_t, axis=X)
            sc = sb.tile([P, 1], f32)
            nc.vector.tensor_scalar_max(out=sc[:], in0=cnt[:], scalar1=1.0)
            rc = sb.tile([P, 1], f32)
            nc.vector.reciprocal(out=rc[:], in_=sc[:])
            valid = sb.tile([P, 1], f32)
            nc.vector.tensor_scalar(
                out=valid[:], in0=cnt[:], scalar1=0.5, scalar2=None, op0=op.is_gt
            )

            # rhs for the per-image reduction matmul:
            # cols 0:30 mean*onehot | 30:60 valid*onehot | 60 pvpp | 61 valid
            rhs1 = sb.tile([P, 2 * M + 2], f32)
            nc.vector.tensor_tensor(
                out=rhs1[:, M:2 * M], in0=oneh_t,
                in1=valid[:].to_broadcast([P, M]), op=op.mult,
            )
            nc.vector.tensor_copy(out=rhs1[:, 2 * M + 1:2 * M + 2], in_=valid[:])

            # ---- per-person partial sums, grouped so DVE work overlaps
            # the remaining gathers ----
            gv = sb.tile([P, K], f32)
            scr = sb.tile([P, K], f32)
            s1c = sb.tile([P, len(GROUPS)], f32)
            s2c = sb.tile([P, len(GROUPS)], f32)
            for i, (a, b) in enumerate(GROUPS):
                nc.vector.tensor_mul(out=gv[:, a:b], in0=g[:, a:b], in1=vis_t[:, a:b])
                nc.vector.reduce_sum(out=s1c[:, i:i + 1], in_=gv[:, a:b], axis=X)
                nc.vector.tensor_mul(out=scr[:, a:b], in0=g[:, a:b], in1=gv[:, a:b])
                nc.vector.reduce_sum(out=s2c[:, i:i + 1], in_=scr[:, a:b], axis=X)
            s1 = sb.tile([P, 1], f32)
            nc.vector.reduce_sum(out=s1[:], in_=s1c[:], axis=X)
            s2 = sb.tile([P, 1], f32)
            nc.vector.reduce_sum(out=s2[:], in_=s2c[:], axis=X)

            mean = sb.tile([P, 1], f32)
            nc.vector.tensor_mul(out=mean[:], in0=s1[:], in1=rc[:])
            mean2 = sb.tile([P, 1], f32)
            nc.vector.tensor_mul(out=mean2[:], in0=mean[:], in1=mean[:])
            ppraw = sb.tile([P, 1], f32)
            nc.vector.tensor_scalar(
                out=ppraw[:], in0=s2[:], scalar1=rc[:], scalar2=None, op0=op.mult
            )
            # pvpp = (ppraw - mean^2) * valid
            nc.vector.tensor_scalar(
                out=rhs1[:, 2 * M:2 * M + 1], in0=ppraw[:],
                scalar1=mean2[:], scalar2=valid[:],
                op0=op.subtract, op1=op.mult,
            )
            nc.vector.tensor_tensor(
                out=rhs1[:, 0:M], in0=oneh_t,
                in1=mean[:].to_broadcast([P, M]), op=op.mult,
            )

            # per-image sums: m1[b, :] = sum_p sel[p, b] * rhs1[p, :]
            m1 = ps.tile([BL, 2 * M + 2], f32)
            nc.tensor.matmul(out=m1[:], lhsT=sel_t[:], rhs=rhs1[:], start=True, stop=True)
            p1s = sb.tile([BL, 2 * M + 2], f32)
            nc.vector.tensor_copy(out=p1s[:], in_=m1[:])

            # broadcast means/valids of each image back to person partitions
            m2 = ps.tile([P, 2 * M], f32)
            nc.tensor.matmul(
                out=m2[:], lhsT=selT_t[:], rhs=p1s[:, 0:2 * M], start=True, stop=True
            )

            # pairwise push term with additive penalty mask:
            # x = (mean_j - mean_i)^2 + 200*(1 - v_i v_j); exp(-x) row-summed
            pen = sb.tile([P, M], f32)
            nc.vector.tensor_scalar(
                out=pen[:], in0=m2[:, M:2 * M],
                scalar1=-200.0, scalar2=200.0, op0=op.mult, op1=op.add,
            )
            d = sb.tile([P, M], f32)
            nc.vector.tensor_tensor(
                out=d[:], in0=m2[:, 0:M],
                in1=mean[:].to_broadcast([P, M]), op=op.subtract,
            )
            d2 = sb.tile([P, M], f32)
            nc.vector.tensor_mul(out=d2[:], in0=d[:], in1=d[:])
            x = sb.tile([P, M], f32)
            nc.vector.tensor_add(out=x[:], in0=d2[:], in1=pen[:])
            e = sb.tile([P, M], f32)
            rowsum = sb.tile([P, 1], f32)
            nc.scalar.activation(
                out=e[:], in_=x[:],
                func=mybir.ActivationFunctionType.Exp,
                bias=zero_c, scale=-1.0,
                accum_out=rowsum[:],
            )
            rsv = sb.tile([P, 1], f32)
            nc.vector.tensor_mul(out=rsv[:], in0=rowsum[:], in1=valid[:])

            m3 = ps.tile([BL, 1], f32)
            nc.tensor.matmul(out=m3[:], lhsT=sel_t[:], rhs=rsv[:], start=True, stop=True)

            # ---- final per-image scalars on partitions 0..3 ----
            nn = p1s[:, 2 * M + 1:2 * M + 2]
            pn = p1s[:, 2 * M:2 * M + 1]
            outt = sb.tile([BL, 2], f32)

            nd = sb.tile([BL, 2], f32)
            nc.vector.tensor_copy(out=nd[:, 0:1], in_=nn)
            nc.vector.tensor_scalar(
                out=nd[:, 1:2], in0=nn, scalar1=nn, scalar2=nn,
                op0=op.mult, op1=op.subtract,
            )
            ndm = sb.tile([BL, 2], f32)
            nc.vector.tensor_scalar_max(out=ndm[:], in0=nd[:], scalar1=1.0)
            rr = sb.tile([BL, 2], f32)
            nc.vector.reciprocal(out=rr[:], in_=ndm[:])
            # pull = pull_num / max(n, 1)
            nc.vector.tensor_scalar(
                out=outt[:, 1:2], in0=pn, scalar1=rr[:, 0:1], scalar2=None,
                op0=op.mult,
            )
            smn = sb.tile([BL, 1], f32)
            nc.vector.tensor_sub(out=smn[:], in0=m3[:], in1=nn)
            # push = (S - n) / max(n^2 - n, 1) * 0.5
            nc.vector.tensor_scalar(
                out=outt[:, 0:1], in0=smn[:], scalar1=rr[:, 1:2], scalar2=0.5,
                op0=op.mult, op1=op.mult,
            )

            nc.sync.dma_start(out=out_d[:], in_=outt[:])

    nc.compile()
    return nc


def _in_maps(tags, joints):
    sel, selT, oneh = _constants()
    tags = np.ascontiguousarray(np.asarray(tags, dtype=np.float32)).reshape(B, N)
    joints = np.asarray(joints, dtype=np.int32)
    idx_all = joints[..., 0]                               # [B, M, K]
    vis_all = (joints[..., 1] > 0).astype(np.float32)      # [B, M, K]

    in_maps = []
    for c in range(NCORES):
        b0 = c * BL
        packed = np.zeros((P, W_PACK), np.float32)
        idx_l = np.zeros((P, K), np.int32)
        for b in range(BL):
            rows = slice(b * M, (b + 1) * M)
            idx_l[rows] = idx_all[b0 + b] + b * N
            packed[rows, C_VIS:C_VIS + K] = vis_all[b0 + b]
        packed[:, C_SEL:C_SEL + BL] = sel
        packed[:, C_ONEH:C_ONEH + M] = oneh
        in_maps.append({
            "tags": np.ascontiguousarray(tags[b0:b0 + BL].reshape(BL * N, 1)),
            "idx": idx_l,
            "packed": packed,
            "selT": selT,
        })
    return in_maps


def _run(in_maps, trace=False):
    from concourse import bass_utils

    if "nc" not in _cache:
        _cache["nc"] = _build()
    return bass_utils.run_bass_kernel_spmd(
        _cache["nc"], in_maps, core_ids=list(range(NCORES)), trace=trace
    )


def kernel(tags, joints):
    res = _run(_in_maps(tags, joints))
    outs = [res.results[c]["out"] for c in range(NCORES)]
    push = np.concatenate([o[:, 0] for o in outs]).astype(np.float32)
    pull = np.concatenate([o[:, 1] for o in outs]).astype(np.float32)
    return push, pull


# revision 21
# speedup vs baseline: 1.1737x; 1.1737x over previous
"""AELoss (associative-embedding push/pull loss) on 8 TRN2 NeuronCores.

Strategy: data-parallel over batch — each of the 8 cores handles 4 images.
The tags tensor is huge ([B, N, 1], N = 17*256*256) but only the visible
(person, joint) pairs are ever needed (vis masks the rest), so the host
compacts just those ~1020 indices per core into [128, C] slots (C ~ 9) and
the kernel issues ONE [128,1] indirect DMA per slot column — the per-
instruction cost of the SWDGE indirect gather is fixed (~1.45us), so
halving the column count nearly halves the gather block.

Per-person sums are recovered from the packed layout with one tiny PE
matmul per column against a host-built one-hot slot->person matrix,
accumulated in PSUM ([128,2] = [sum g, sum g^2] per person) — all hidden
under the remaining gathers. Per-image sums then go through a second
selection matmul, the pairwise push term exp(-(mi-mj)^2) uses the ACT
engine with the pair-validity mask folded in as an additive -200 penalty
(via the matmul and the activation bias), and accum_out provides the row
sums. Output [4, 2] = (push, pull) per image per core.

Identities used (exact against the reference):
  pull_pp = sum(g^2 v)/safe_cnt - mean^2
  pull    = pull_num / max(n,1)        (n>0 guard redundant: n=0 -> 0)
  push    = (S - n)/max(n^2-n, 1)*0.5  (n<2 guard redundant: S=n for n<=1)
"""

import numpy as np

B, M, K = 32, 30, 17
N = 17 * 256 * 256
NCORES = 8
BL = B // NCORES          # images per core
P = 128
PERS = BL * M             # persons per core (120)

# aux input layout (f32 columns): vis | sel | oneh | -200 bias col
C_VIS = 0
C_SEL = K                 # [17, 21)
C_ONEH = K + BL           # [21, 51)
C_BIAS = C_ONEH + M       # [51, 52)
W_AUX = C_BIAS + 1        # 52

_cache = {}


def _constants():
    p = np.arange(P)
    m = p < PERS
    sel = np.zeros((P, BL), np.float32)
    sel[p[m], (p // M)[m]] = 1.0
    selT = np.ascontiguousarray(sel.T)             # [BL, P]
    oneh = np.zeros((P, M), np.float32)
    oneh[p[m], (p % M)[m]] = 1.0
    return sel, selT, oneh


def _strip_init_barrier(nc):
    """Drop the Bass-init const-AP memsets and the all-engine barrier that
    orders them — nothing in this kernel reads the const APs (activation
    bias is passed as an explicit AP)."""
    import concourse.mybir as mybir

    bb = nc.main_func.blocks[0]
    drop = set()
    for ins in bb.instructions:
        if isinstance(ins, (mybir.InstMemset, mybir.InstDrain, mybir.InstEventSemaphore)):
            drop.add(ins.name)
    if not drop:
        return
    keep = [ins for ins in bb.instructions if ins.name not in drop]
    del bb.instructions[:]
    for ins in keep:
        bb.add_instruction(ins)


def _build(C):
    import concourse.bass as bass
    import concourse.bacc as bacc
    import concourse.mybir as mybir
    from concourse.tile import TileContext

    f32 = mybir.dt.float32
    i32 = mybir.dt.int32
    X = mybir.AxisListType.X
    op = mybir.AluOpType

    nc = bacc.Bacc(trn_type="TRN2")
    _strip_init_barrier(nc)
    tags_d = nc.dram_tensor("tags", [BL * N, 1], f32, kind="ExternalInput")
    idx_d = nc.dram_tensor("idx", [P, C], i32, kind="ExternalInput")
    amat_d = nc.dram_tensor("amat", [P, C * P], f32, kind="ExternalInput")
    aux_d = nc.dram_tensor("aux", [P, W_AUX], f32, kind="ExternalInput")
    selT_d = nc.dram_tensor("selT", [BL, P], f32, kind="ExternalInput")
    out_d = nc.dram_tensor("out", [BL, 2], f32, kind="ExternalOutput")

    with TileContext(nc) as tc:
        with (
            tc.tile_pool(name="sb", bufs=1) as sb,
            tc.tile_pool(name="ps", bufs=1, space="PSUM") as ps,
        ):
            # idx first, on its own HWDGE path, so the gathers start ASAP
            idx_t = sb.tile([P, C], i32)
            nc.sync.dma_start(out=idx_t[:], in_=idx_d[:])

            # gathers: one [128,1] indirect DMA per packed slot column
            gg = sb.tile([P, 2 * C], f32)
            for c in range(C):
                nc.gpsimd.indirect_dma_start(
                    out=gg[:, 2 * c:2 * c + 1],
                    out_offset=None,
                    in_=tags_d[:],
                    in_offset=bass.IndirectOffsetOnAxis(ap=idx_t[:, c:c + 1], axis=0),
                    oob_is_err=False,
                    bounds_check=BL * N - 1,
                )

            # remaining small inputs on the scalar engine's HWDGE queue
            amat_t = sb.tile([P, C * P], f32)
            nc.scalar.dma_start(out=amat_t[:], in_=amat_d[:])
            aux_t = sb.tile([P, W_AUX], f32)
            nc.scalar.dma_start(out=aux_t[:], in_=aux_d[:])
            vis_t = aux_t[:, C_VIS:C_VIS + K]
            oneh_t = aux_t[:, C_ONEH:C_ONEH + M]
            bias_c = aux_t[:, C_BIAS:C_BIAS + 1]
            selT_raw = sb.tile([BL, P], f32)
            nc.scalar.dma_start(out=selT_raw[:], in_=selT_d[:])

            # per-person [sum g, sum g^2] via one accumulating matmul per
            # column: square each landed column, matmul against the one-hot
            # slot->person matrix (overlaps the remaining gathers)
            ps12 = ps.tile([P, 2], f32)
            for c in range(C):
                nc.vector.tensor_mul(
                    out=gg[:, 2 * c + 1:2 * c + 2],
                    in0=gg[:, 2 * c:2 * c + 1], in1=gg[:, 2 * c:2 * c + 1],
                )
                nc.tensor.matmul(
                    out=ps12[:], lhsT=amat_t[:, c * P:(c + 1) * P],
                    rhs=gg[:, 2 * c:2 * c + 2],
                    start=(c == 0), stop=(c == C - 1),
                )

            # gather-independent prep (runs in the gather window)
            sel_t = sb.tile([P, BL], f32)
            nc.vector.tensor_copy(out=sel_t[:], in_=aux_t[:, C_SEL:C_SEL + BL])
            selT_t = sb.tile([BL, P], f32)
            nc.vector.tensor_copy(out=selT_t[:], in_=selT_raw[:])

            cnt = sb.tile([P, 1], f32)
            nc.vector.reduce_sum(out=cnt[:], in_=vis_t, axis=X)
            sc = sb.tile([P, 1], f32)
            nc.vector.tensor_scalar_max(out=sc[:], in0=cnt[:], scalar1=1.0)
            rc = sb.tile([P, 1], f32)
            nc.vector.reciprocal(out=rc[:], in_=sc[:])
            valid = sb.tile([P, 1], f32)
            nc.vector.tensor_scalar(
                out=valid[:], in0=cnt[:], scalar1=0.5, scalar2=None, op0=op.is_gt
            )
            v200 = sb.tile([P, 1], f32)
            nc.vector.tensor_scalar_mul(out=v200[:], in0=valid[:], scalar1=-200.0)

            # rhs for the per-image reduction matmul:
            # cols 0:30 mean*onehot | 30:60 -200*valid*onehot | 60 pvpp | 61 valid
            rhs1 = sb.tile([P, 2 * M + 2], f32)
            nc.vector.tensor_tensor(
                out=rhs1[:, M:2 * M], in0=oneh_t,
                in1=v200[:].to_broadcast([P, M]), op=op.mult,
            )
            nc.vector.tensor_copy(out=rhs1[:, 2 * M + 1:2 * M + 2], in_=valid[:])

            # ---- post-gather chain ----
            s12 = sb.tile([P, 2], f32)
            nc.vector.tensor_copy(out=s12[:], in_=ps12[:])

            mean = sb.tile([P, 1], f32)
            nc.vector.tensor_scalar(
                out=mean[:], in0=s12[:, 0:1], scalar1=rc[:], scalar2=None, op0=op.mult
            )
            mean2 = sb.tile([P, 1], f32)
            nc.vector.tensor_mul(out=mean2[:], in0=mean[:], in1=mean[:])
            ppraw = sb.tile([P, 1], f32)
            nc.vector.tensor_scalar(
                out=ppraw[:], in0=s12[:, 1:2], scalar1=rc[:], scalar2=None, op0=op.mult
            )
            # pvpp = (ppraw - mean^2) * valid
            nc.vector.tensor_scalar(
                out=rhs1[:, 2 * M:2 * M + 1], in0=ppraw[:],
                scalar1=mean2[:], scalar2=valid[:],
                op0=op.subtract, op1=op.mult,
            )
            nc.vector.tensor_tensor(
                out=rhs1[:, 0:M], in0=oneh_t,
                in1=mean[:].to_broadcast([P, M]), op=op.mult,
            )

            # per-image sums: m1[b, :] = sum_p sel[p, b] * rhs1[p, :]
            m1 = ps.tile([BL, 2 * M + 2], f32)
            nc.tensor.matmul(out=m1[:], lhsT=sel_t[:], rhs=rhs1[:], start=True, stop=True)
            p1s = sb.tile([BL, 2 * M + 2], f32)
            nc.vector.tensor_copy(out=p1s[:], in_=m1[:])

            # broadcast means (and -200*valid) of each image back to persons
            m2 = ps.tile([P, 2 * M], f32)
            nc.tensor.matmul(
                out=m2[:], lhsT=selT_t[:], rhs=p1s[:, 0:2 * M], start=True, stop=True
            )

            # pairwise push term; pair mask folded in additively:
            # e = exp(-(d^2 + m2_v200col) - 200) = exp(-d^2) iff v_j else ~0
            d = sb.tile([P, M], f32)
            nc.vector.tensor_tensor(
                out=d[:], in0=m2[:, 0:M],
                in1=mean[:].to_broadcast([P, M]), op=op.subtract,
            )
            d2 = sb.tile([P, M], f32)
            nc.vector.tensor_mul(out=d2[:], in0=d[:], in1=d[:])
            x = sb.tile([P, M], f32)
            nc.vector.tensor_add(out=x[:], in0=d2[:], in1=m2[:, M:2 * M])
            e = sb.tile([P, M], f32)
            rowsum = sb.tile([P, 1], f32)
            nc.scalar.activation(
                out=e[:], in_=x[:],
                func=mybir.ActivationFunctionType.Exp,
                bias=bias_c, scale=-1.0,
                accum_out=rowsum[:],
            )
            rsv = sb.tile([P, 1], f32)
            nc.vector.tensor_mul(out=rsv[:], in0=rowsum[:], in1=valid[:])

            m3 = ps.tile([BL, 1], f32)
            nc.tensor.matmul(out=m3[:], lhsT=sel_t[:], rhs=rsv[:], start=True, stop=True)

            # ---- final per-image scalars on partitions 0..3 ----
            nn = p1s[:, 2 * M + 1:2 * M + 2]
            pn = p1s[:, 2 * M:2 * M + 1]
            outt = sb.tile([BL, 2], f32)

            nd = sb.tile([BL, 2], f32)
            nc.vector.tensor_copy(out=nd[:, 0:1], in_=nn)
            nc.vector.tensor_scalar(
                out=nd[:, 1:2], in0=nn, scalar1=nn, scalar2=nn,
                op0=op.mult, op1=op.subtract,
            )
            ndm = sb.tile([BL, 2], f32)
            nc.vector.tensor_scalar_max(out=ndm[:], in0=nd[:], scalar1=1.0)
            rr = sb.tile([BL, 2], f32)
            nc.vector.reciprocal(out=rr[:], in_=ndm[:])
            # pull = pull_num / max(n, 1)
            nc.vector.tensor_scalar(
                out=outt[:, 1:2], in0=pn, scalar1=rr[:, 0:1], scalar2=None,
                op0=op.mult,
            )
            smn = sb.tile([BL, 1], f32)
            nc.vector.tensor_sub(out=smn[:], in0=m3[:], in1=nn)
            # push = (S - n) / max(n^2 - n, 1) * 0.5
            nc.vector.tensor_scalar(
                out=outt[:, 0:1], in0=smn[:], scalar1=rr[:, 1:2], scalar2=0.5,
                op0=op.mult, op1=op.mult,
            )

            nc.sync.dma_start(out=out_d[:], in_=outt[:])

    nc.compile()
    return nc


def _in_maps(tags, joints):
    sel, selT, oneh = _constants()
    tags = np.ascontiguousarray(np.asarray(tags, dtype=np.float32)).reshape(B, N)
    joints = np.asarray(joints, dtype=np.int32)
    idx_all = joints[..., 0]                               # [B, M, K]
    vis_all = joints[..., 1] > 0                           # [B, M, K] bool

    # compact visible (person, joint) slots per core
    per_core = []
    C = 1
    for c in range(NCORES):
        b0 = c * BL
        persons = []
        fidx = []
        for b in range(BL):
            vb = vis_all[b0 + b]                           # [M, K]
            mm, kk = np.nonzero(vb)
            persons.append(b * M + mm)
            fidx.append(idx_all[b0 + b][mm, kk] + b * N)
        persons = np.concatenate(persons)
        fidx = np.concatenate(fidx)
        per_core.append((persons, fidx))
        C = max(C, (len(fidx) + P - 1) // P)

    in_maps = []
    for c in range(NCORES):
        b0 = c * BL
        persons, fidx = per_core[c]
        n_slots = len(fidx)
        idx_l = np.zeros((P, C), np.int32)
        amat = np.zeros((P, C * P), np.float32)
        s = np.arange(n_slots)
        sp, scol = s % P, s // P
        idx_l[sp, scol] = fidx
        amat[sp, scol * P + persons] = 1.0

        aux = np.zeros((P, W_AUX), np.float32)
        for b in range(BL):
            rows = slice(b * M, (b + 1) * M)
            aux[rows, C_VIS:C_VIS + K] = vis_all[b0 + b]
        aux[:, C_SEL:C_SEL + BL] = sel
        aux[:, C_ONEH:C_ONEH + M] = oneh
        aux[:, C_BIAS] = -200.0
        in_maps.append({
            "tags": np.ascontiguousarray(tags[b0:b0 + BL].reshape(BL * N, 1)),
            "idx": idx_l,
            "amat": amat,
            "aux": aux,
            "selT": selT,
        })
    return C, in_maps


def _run(C, in_maps, trace=False):
    from concourse import bass_utils

    if C not in _cache:
        _cache[C] = _build(C)
    return bass_utils.run_bass_kernel_spmd(
        _cache[C], in_maps, core_ids=list(range(NCORES)), trace=trace
    )


def kernel(tags, joints):
    C, in_maps = _in_maps(tags, joints)
    res = _run(C, in_maps)
    outs = [res.results[c]["out"] for c in range(NCORES)]
    push = np.concatenate([o[:, 0] for o in outs]).astype(np.float32)
    pull = np.concatenate([o[:, 1] for o in outs]).astype(np.float32)
    return push, pull
